# revision 13
# baseline (speedup 1.0000x reference)
"""Trainium2 Bass kernel for GQA causal attention (nn_Attention).

Reference computation (B=2, S=2048, D=4096, H=32, KV=8, HD=128):
    q/k/v projections -> RoPE(q, k) -> GQA attention with additive mask
    -> softmax -> out projection.

Sharding: TP=4 over heads x DP=2 over batch on 8 NeuronCores.
Each core computes, for its batch b and head shard tp:
    Q^T = (x_b @ wq_tp)^T, K^T, V  (projections with RoPE folded via
    host-side even/odd weight-column reordering + on-device rotation)
    S^T = K^T . Q^T per head (scores, transposed layout)
    P^T = exp(S^T - 3) * expmask_tile  (lazy softmax; the -3 shift keeps
          exp and its row sums inside fp16 range and cancels in the
          normalize step)
    U^T = V^T-accumulated P^T; rowsums via DVE tile accumulation + one
          M=128 ones-matmul per (head, chunk) so the per-query reciprocal
          is broadcast-free
    att^T = U^T * (1/rowsum);  out_partial = att @ wo_tp
Host sums the 4 TP partials per batch (the row-parallel all-reduce).

All tensors are fp16 (better mantissa than bf16 at equal speed); matmuls
accumulate in fp32 PSUM. Score tiles are packed in pairs into [128,1024]
fp32 PSUM slots so one ACTIVATE covers up to 1024 columns. Phase B is
software-pipelined one head ahead (scores+exp for head h interleave with
the AV matmuls of head h-1) so ScalarE exp latency never stalls TensorE,
and the out-projection of the previous query chunk fills the pipeline
edges.
"""

import os
import math
import numpy as np

# ---------------------------------------------------------------- constants
B, S, D = 2, 2048, 4096
H, KV, HD = 32, 8, 128
N_REP = H // KV
TP, DP = 4, 2
N_CORES = TP * DP
HL = H // TP            # 8 local q heads
KVL = KV // TP          # 2 local kv heads
P = 128                 # partitions
KT = D // P             # 32 contraction tiles for projections
NJ_FULL = S // 512      # 4 seq chunks of 512
NST = S // P            # 16 seq tiles of 128
HF = np.float16
PSUM_SLOT = 1024        # fp32 elements per 2-bank psS slot
PSUM_BANK = 512         # fp32 elements per PSUM bank
EXP_SHIFT = -3.0        # exp(s + EXP_SHIFT): keeps pt/rowsums in fp16 range

# module-level handle for test harness introspection
last_results = None
_cache = {}


def _classify_mask(mask: np.ndarray):
    """Turn the additive mask into multiplicative per-tile factors.

    Returns (table, uniq) where table[i][j] is 'full' (factor==1
    everywhere), 'zero' (factor==0 everywhere -> tile skipped), or an
    index into uniq, the list of distinct [128,512] f32 factor tiles in
    S^T layout ([sk, sq]).
    """
    m = mask.astype(np.float64)
    rowmax = np.max(m, axis=1, keepdims=True)  # per-query max over keys
    rowmax = np.where(np.isfinite(rowmax), rowmax, 0.0)
    em = np.exp(m - rowmax)                    # [sq, sk] in [0, inf)
    emT = np.ascontiguousarray(em.T).astype(np.float32)  # [sk, sq]
    table = [[None] * NJ_FULL for _ in range(NST)]
    uniq = []
    keys = {}
    for j in range(NJ_FULL):
        first = True
        for i in range(NST):
            t = emT[i * P:(i + 1) * P, j * 512:(j + 1) * 512]
            if np.all(t == 1.0):
                table[i][j] = "full"
                first = False
                continue
            if np.all(t == 0.0):
                table[i][j] = "zero"
                continue
            cols1 = np.all(t == 1.0, axis=0)   # all-ones columns
            cols0 = np.all(t == 0.0, axis=0)   # all-zero columns
            # live range starts after leading all-zero cols (first tile of a
            # j-chunk must start at 0 so the PSUM bank is fully initialized)
            lo = 0
            if not first:
                while lo < 512 and cols0[lo]:
                    lo += 1
            hi = 512
            while hi > lo and cols1[hi - 1]:
                hi -= 1
            w = hi - lo
            sub = t[:, lo:hi]
            key = sub.tobytes()
            if key not in keys:
                keys[key] = len(uniq)
                pad = np.ones((P, 512), np.float32)
                pad[:, :w] = sub
                uniq.append(pad)
            table[i][j] = (lo, w, keys[key])
            first = False
    return table, uniq


def _rope_perm(n_heads):
    """Column permutation putting even rope dims first, odd second, per head."""
    perm = []
    for h in range(n_heads):
        perm += [h * HD + 2 * i for i in range(HD // 2)]
        perm += [h * HD + 2 * i + 1 for i in range(HD // 2)]
    return np.array(perm, dtype=np.int64)


def _pack_groups(tiles):
    """Pack (i, lo) score tiles into [P, PSUM_SLOT] fp32 slots.

    Each tile occupies width 512-lo; a tile may not cross a PSUM bank
    boundary (matmul output must stay within one bank). Returns a list of
    groups; each group is (span, [(i, lo, off), ...]).
    """
    groups = []
    cur = []
    off = 0
    for i, lo in tiles:
        w = 512 - lo
        noff = off
        if (noff // PSUM_BANK) != ((noff + w - 1) // PSUM_BANK):
            noff = ((noff // PSUM_BANK) + 1) * PSUM_BANK
        if noff + w > PSUM_SLOT:
            groups.append((off, cur))
            cur = []
            noff = 0
        cur.append((i, lo, noff))
        off = noff + w
    if cur:
        groups.append((off, cur))
    return groups


def _build(table_sig, table, n_uniq):
    """Build + compile the SPMD Bass program for one mask classification."""
    import concourse.bass as bass
    import concourse.tile as tile
    import concourse.mybir as mybir
    from concourse import bacc

    hf = mybir.dt.float16
    f32 = mybir.dt.float32
    Exp = mybir.ActivationFunctionType.Exp
    MULT = mybir.AluOpType.mult
    ADD = mybir.AluOpType.add

    nc = bacc.Bacc("TRN2", target_bir_lowering=False, debug=False,
                   enable_asserts=False, num_devices=N_CORES)

    xT_d = nc.dram_tensor("xT", [D, S], hf, kind="ExternalInput")
    wq_d = nc.dram_tensor("wq", [D, HL * HD], hf, kind="ExternalInput")
    wk_d = nc.dram_tensor("wk", [D, KVL * HD], hf, kind="ExternalInput")
    wv_d = nc.dram_tensor("wv", [D, KVL * HD], hf, kind="ExternalInput")
    wo_d = nc.dram_tensor("wo", [HL * HD, D], hf, kind="ExternalInput")
    cosf_d = nc.dram_tensor("cosf", [P, S], hf, kind="ExternalInput")
    ssf_d = nc.dram_tensor("ssf", [P, S], hf, kind="ExternalInput")
    ones_d = nc.dram_tensor("ones", [P, P], hf, kind="ExternalInput")
    em_d = [nc.dram_tensor(f"em{u}", [P, 512], hf, kind="ExternalInput")
            for u in range(n_uniq)]
    out_d = nc.dram_tensor("out", [S, D], hf, kind="ExternalOutput")

    with tile.TileContext(nc) as tc:
        with tc.tile_pool(name="consts", bufs=1) as cpool:
            cosf = cpool.tile([P, S], hf, tag="cosf", name="cosf")
            ssf = cpool.tile([P, S], hf, tag="ssf", name="ssf")
            ones = cpool.tile([P, P], hf, tag="ones", name="ones")
            shift = cpool.tile([P, 1], f32, tag="shift", name="shift")
            nc.vector.memset(shift[:], EXP_SHIFT)
            # preload the exp table set during the startup DMA wait so the
            # first real softmax exp doesn't pay the ~2.7us table load
            warm = cpool.tile([P, 1], hf, tag="warm", name="warm")
            nc.scalar.activation(warm[:], shift[:], Exp)
            em_sb = [cpool.tile([P, 512], hf, tag=f"em{u}", name=f"em{u}")
                     for u in range(n_uniq)]

            def load_consts():
                # issued behind the first critical weight loads so they do
                # not delay the first matmul
                nc.sync.dma_start(cosf[:], cosf_d[:, :])
                nc.sync.dma_start(ssf[:], ssf_d[:, :])
                nc.sync.dma_start(ones[:], ones_d[:, :])
                for u in range(n_uniq):
                    nc.sync.dma_start(em_sb[u][:], em_d[u][:, :])

            qkv_pool = tc.alloc_tile_pool(name="qkv", bufs=1)
            QT = [qkv_pool.tile([P, S], hf, tag=f"qt{h}", name=f"qt{h}") for h in range(HL)]
            KTt = [qkv_pool.tile([P, S], hf, tag=f"kt{g}", name=f"kt{g}") for g in range(KVL)]
            V = [qkv_pool.tile([P, KVL * HD], hf, tag=f"v{st}", name=f"v{st}") for st in range(NST)]

            # ------------- phase A: projections + RoPE ------------
            XH = 2 if NJ_FULL % 2 == 0 else 1
            SH = S // XH
            QG = 2 if HL % 2 == 0 else HL   # q heads per weight group
            with tc.tile_pool(name="xt", bufs=1) as xt_pool, \
                 tc.tile_pool(name="wq", bufs=2) as wq_pool, \
                 tc.tile_pool(name="wk", bufs=1) as wk_pool, \
                 tc.tile_pool(name="wv", bufs=1) as wv_pool, \
                 tc.tile_pool(name="ropetmp", bufs=2) as rt_pool, \
                 tc.tile_pool(name="psA", bufs=3, space="PSUM") as psA, \
                 tc.tile_pool(name="psV", bufs=2, space="PSUM") as psV:
                for half in range(XH):
                    s0 = half * SH
                    # V weights first so the first matmul can start as soon
                    # as the first xt column chunk lands.
                    wvt = []
                    for k in range(KT):
                        t = wv_pool.tile([P, KVL * HD], hf, tag=f"wv{k}", name=f"wv{k}")
                        nc.sync.dma_start(t[:], wv_d[k * P:(k + 1) * P, :])
                        wvt.append(t)
                    xt = []
                    for k in range(KT):
                        t = xt_pool.tile([P, SH], hf, tag=f"xt{k}", name=f"xt{k}")
                        xt.append(t)
                    # column-chunked loads: compute on chunk c may start
                    # while chunk c+1 is still in flight (finer first chunks
                    # so the first V matmul starts sooner)
                    chunks = [(0, 256), (256, 256)]
                    c = chunks[-1][0] + chunks[-1][1]
                    while c < SH:
                        chunks.append((c, 512))
                        c += 512
                    for c, cw in chunks:
                        for k in range(KT):
                            nc.gpsimd.dma_start(
                                xt[k][:, c:c + cw],
                                xT_d[k * P:(k + 1) * P, s0 + c:s0 + c + cw])

                    def rope_gen(dst, wt, coff, jj):
                        """dst[:, jj*512..] = rope((x @ w)[:, coff:coff+128])"""
                        ps = psA.tile([P, 512], f32, tag="psqk", name="psqk")
                        lo = jj * 512 - s0
                        for k in range(KT):
                            nc.tensor.matmul(ps[:], wt[k][:, coff:coff + P],
                                             xt[k][:, lo:lo + 512],
                                             start=(k == 0), stop=(k == KT - 1))
                        qb = rt_pool.tile([P, 512], hf, tag="qb", name="qb")
                        nc.scalar.copy(qb[:], ps[:])
                        qsw = rt_pool.tile([P, 512], hf, tag="qsw", name="qsw")
                        nc.scalar.copy(qsw[0:64, :], ps[64:128, :])
                        nc.scalar.copy(qsw[64:128, :], ps[0:64, :])
                        t1 = rt_pool.tile([P, 512], hf, tag="t1", name="t1")
                        nc.vector.tensor_tensor(
                            t1[:], qb[:], cosf[:, jj * 512:jj * 512 + 512], MULT)
                        t2 = rt_pool.tile([P, 512], hf, tag="t2", name="t2")
                        nc.vector.tensor_tensor(
                            t2[:], qsw[:], ssf[:, jj * 512:jj * 512 + 512], MULT)
                        nc.vector.tensor_tensor(
                            dst[:, jj * 512:jj * 512 + 512], t1[:], t2[:], ADD)

                    half_js = list(range(half * (NJ_FULL // XH),
                                         (half + 1) * (NJ_FULL // XH)))
                    # V and K first so attention can start early
                    for st in range(half * (NST // XH), (half + 1) * (NST // XH)):
                        ps = psV.tile([P, KVL * HD], f32, tag="psv", name="psv")
                        lo = st * P - s0
                        for k in range(KT):
                            nc.tensor.matmul(ps[:], xt[k][:, lo:lo + P], wvt[k][:],
                                             start=(k == 0), stop=(k == KT - 1))
                        nc.scalar.copy(V[st][:], ps[:])
                    wkt = []
                    for k in range(KT):
                        t = wk_pool.tile([P, KVL * HD], hf, tag=f"wk{k}", name=f"wk{k}")
                        nc.sync.dma_start(t[:], wk_d[k * P:(k + 1) * P, :])
                        wkt.append(t)
                    if half == 0:
                        load_consts()
                    for g in range(KVL):
                        for jj in half_js:
                            rope_gen(KTt[g], wkt, g * HD, jj)
                    for hg in range(HL // QG):
                        wqt = []
                        for k in range(KT):
                            t = wq_pool.tile([P, QG * HD], hf, tag=f"wq{k}", name=f"wq{k}")
                            nc.sync.dma_start(
                                t[:], wq_d[k * P:(k + 1) * P,
                                           hg * QG * HD:(hg + 1) * QG * HD])
                            wqt.append(t)
                        for h in range(hg * QG, (hg + 1) * QG):
                            for jj in half_js:
                                rope_gen(QT[h], wqt, (h - hg * QG) * HD, jj)

            # ------------- phase B+C: attention + out projection ------
            ut_pool = tc.alloc_tile_pool(name="ut", bufs=1)
            UT = [ut_pool.tile([P, S], hf, tag=f"ut{h}", name=f"ut{h}")
                  for h in range(HL)]
            wo_pool = tc.alloc_tile_pool(name="wo", bufs=1)
            ob_pool = tc.alloc_tile_pool(name="ob", bufs=2)
            wot = []
            for h in range(HL):
                t = wo_pool.tile([P, D], hf, tag=f"wo{h}", name=f"wo{h}")
                nc.sync.dma_start(t[:], wo_d[h * P:(h + 1) * P, :])
                wot.append(t)
            OBH = 2 if D >= 2048 else 1
            OBW = D // OBH
            NDC = OBW // 512
            with tc.tile_pool(name="pt", bufs=10) as pt_pool, \
                 tc.tile_pool(name="acc", bufs=4) as acc_pool, \
                 tc.tile_pool(name="rnorm", bufs=2) as rn_pool, \
                 tc.tile_pool(name="psS", bufs=2, space="PSUM") as psS_pool, \
                 tc.tile_pool(name="psU", bufs=2, space="PSUM") as psU_pool, \
                 tc.tile_pool(name="psR", bufs=2, space="PSUM") as psR_pool:

                def c_chunk_emitters(st):
                    """One closure per (half, dch) out-proj chunk of seq
                    tile st; each runs 8 accumulating matmuls; ob staging
                    and the output DMA are woven into the first/last
                    chunk of each half."""
                    ems = []
                    state = {}
                    for half in range(OBH):
                        for dch in range(NDC):
                            def emit(st=st, half=half, dch=dch):
                                if dch == 0:
                                    state[half] = ob_pool.tile(
                                        [P, OBW], hf, tag="ob", name="ob")
                                ob = state[half]
                                dc = half * NDC + dch
                                psO = psU_pool.tile([P, 512], f32,
                                                    tag="psu", name="psu")
                                for h in range(HL):
                                    nc.tensor.matmul(
                                        psO[:], UT[h][:, st * P:(st + 1) * P],
                                        wot[h][:, dc * 512:dc * 512 + 512],
                                        start=(h == 0), stop=(h == HL - 1))
                                dsl = slice(dch * 512, dch * 512 + 512)
                                if dch % 2 == 0:
                                    nc.vector.tensor_copy(ob[:, dsl], psO[:])
                                else:
                                    nc.scalar.copy(ob[:, dsl], psO[:])
                                if dch == NDC - 1:
                                    nc.gpsimd.dma_start(
                                        out_d[st * P:(st + 1) * P,
                                              half * OBW:(half + 1) * OBW],
                                        ob[:])
                            ems.append(emit)
                    return ems

                for j in range(NJ_FULL):
                    jsl = slice(j * 512, j * 512 + 512)
                    inc = [i for i in range(NST) if table[i][j] != "zero"]
                    tiles = [(i, 0 if table[i][j] == "full" else table[i][j][0])
                             for i in inc]
                    groups = _pack_groups(tiles)
                    ngr = len(groups)
                    # out-proj chunks of the previous query chunk: first
                    # seq tile interleaves into the k=0 pipeline edge,
                    # the rest run after this chunk's head loop.
                    cfill = c_chunk_emitters(4 * (j - 1)) if j > 0 else []
                    per_step = -(-len(cfill) // ngr) if cfill else 0

                    psUs = {}
                    accs = {}
                    pts = {}
                    lag = 2 if j == 0 else 1
                    for k in range(HL + lag):
                        sc_h = k if k < HL else None
                        av_h = k - lag if k >= lag else None
                        for step in range(ngr):
                            span, grp = groups[step]
                            if sc_h is not None:
                                h = sc_h
                                g = h // N_REP
                                psS = psS_pool.tile([P, PSUM_SLOT], f32,
                                                    tag="pss", name="pss")
                                for (i, lo, off) in grp:
                                    nc.tensor.matmul(
                                        psS[:, off:off + 512 - lo],
                                        KTt[g][:, i * P:(i + 1) * P],
                                        QT[h][:, j * 512 + lo:j * 512 + 512],
                                        start=True, stop=True)
                                pt = pt_pool.tile([P, PSUM_SLOT], hf,
                                                  tag="pt", name="pt")
                                pts[(h, step)] = pt
                                nc.scalar.activation(pt[:, 0:span],
                                                     psS[:, 0:span], Exp,
                                                     bias=shift[:])
                                for (i, lo, off) in grp:
                                    cls = table[i][j]
                                    if cls != "full":
                                        _, w, u = cls
                                        nc.vector.tensor_tensor(
                                            pt[:, off:off + w],
                                            pt[:, off:off + w],
                                            em_sb[u][:, 0:w], MULT)
                                # rowsum accumulation on DVE
                                if step == 0:
                                    acc = acc_pool.tile([P, 512], hf,
                                                        tag="acc", name="acc")
                                    accs[h] = acc
                                acc = accs[h]
                                for (i, lo, off) in grp:
                                    if step == 0 and (i, lo, off) == grp[0]:
                                        nc.vector.tensor_copy(
                                            acc[:], pt[:, off:off + 512])
                                    else:
                                        nc.vector.tensor_tensor(
                                            acc[:, lo:], acc[:, lo:],
                                            pt[:, off:off + 512 - lo], ADD)
                            if av_h is not None:
                                h = av_h
                                g = h // N_REP
                                if step == 0:
                                    psUs[h] = psU_pool.tile(
                                        [P, 512], f32, tag="psu", name="psu")
                                psU = psUs[h]
                                ntile = len(tiles)
                                done = sum(len(gr) for _, gr in groups[:step])
                                for (i, lo, off) in grp:
                                    nc.tensor.matmul(
                                        psU[:, lo:],
                                        V[i][:, g * HD:(g + 1) * HD],
                                        pts[(h, step)][:, off:off + 512 - lo],
                                        start=(done == 0),
                                        stop=(done == ntile - 1))
                                    done += 1
                            if av_h is None and cfill:
                                for _ in range(per_step):
                                    if cfill:
                                        cfill.pop(0)()
                        if av_h is not None:
                            h = av_h
                            # partition-reduce the DVE-accumulated rowsums;
                            # M=128 replicates them so no broadcast needed
                            psR = psR_pool.tile([P, 512], f32,
                                                tag="psr", name="psr")
                            nc.tensor.matmul(psR[:], ones[:, 0:P], accs[h][:],
                                             start=True, stop=True)
                            rb = rn_pool.tile([P, 512], f32, tag="rb", name="rb")
                            nc.vector.reciprocal_approx_fast(rb[:], psR[:])
                            nc.vector.tensor_tensor(UT[h][:, jsl],
                                                    psUs[h][:], rb[:], MULT)
                    for emit in cfill:
                        emit()
                    # remaining out-proj seq tiles of the previous chunk
                    if j > 0:
                        for st in range(4 * (j - 1) + 1, 4 * j):
                            for emit in c_chunk_emitters(st):
                                emit()
                # out projection for the final query chunk
                for st in range(4 * (NJ_FULL - 1), NST):
                    for emit in c_chunk_emitters(st):
                        emit()
            ob_pool.release()
            wo_pool.release()
            ut_pool.release()
            qkv_pool.release()

    nc.compile()
    return nc


def kernel(x, freqs_cos, freqs_sin, mask, wq, wk, wv, wo):
    global last_results
    from concourse.bass_utils import run_bass_kernel_spmd

    x = np.asarray(x)
    mask = np.asarray(mask, dtype=np.float32)
    table, uniq = _classify_mask(mask)
    sig = tuple(tuple(r) for r in table), len(uniq)
    key = ("k", sig)
    if key not in _cache:
        _cache[key] = _build(sig, table, len(uniq))
    nc = _cache[key]

    qperm = _rope_perm(H)
    kperm = _rope_perm(KV)
    wq_r = np.asarray(wq)[:, qperm]
    wk_r = (np.asarray(wk) * (1.0 / math.sqrt(HD)))[:, kperm]
    wv_n = np.asarray(wv)
    wo_n = np.asarray(wo)

    cosT = np.asarray(freqs_cos).T.astype(np.float32)     # [64, S]
    sinT = np.asarray(freqs_sin).T.astype(np.float32)
    cosf = np.concatenate([cosT, cosT], axis=0).astype(HF)  # [128, S]
    ssf = np.concatenate([-sinT, sinT], axis=0).astype(HF)
    ones = np.ones((P, P), dtype=HF)

    in_maps = []
    for c in range(N_CORES):
        b, tp = c // TP, c % TP
        m = {
            "xT": np.ascontiguousarray(x[b].T).astype(HF),
            "wq": np.ascontiguousarray(wq_r[:, tp * HL * HD:(tp + 1) * HL * HD]).astype(HF),
            "wk": np.ascontiguousarray(wk_r[:, tp * KVL * HD:(tp + 1) * KVL * HD]).astype(HF),
            "wv": np.ascontiguousarray(wv_n[:, tp * KVL * HD:(tp + 1) * KVL * HD]).astype(HF),
            "wo": np.ascontiguousarray(wo_n[tp * HL * HD:(tp + 1) * HL * HD, :]).astype(HF),
            "cosf": cosf, "ssf": ssf, "ones": ones,
        }
        for u, t in enumerate(uniq):
            m[f"em{u}"] = t.astype(HF)
        in_maps.append(m)

    trace = bool(os.environ.get("BASS_TRACE"))
    try:
        import antenv.axon_hooks  # noqa: F401
    except ImportError:
        # bass_utils re-reads BASS_TRACE internally and would crash on the
        # missing NTFF hook module; fall back to an untraced run instead.
        os.environ.setdefault("BASS_NEVER_TRACE", "1")
        trace = False
    last_results = run_bass_kernel_spmd(
        nc, in_maps, core_ids=list(range(N_CORES)), trace=trace)

    out = np.zeros((B, S, D), dtype=np.float32)
    for c in range(N_CORES):
        out[c // TP] += last_results.results[c]["out"]
    return out


# revision 14
# speedup vs baseline: 1.1867x; 1.1867x over previous
"""Trainium2 Bass kernel for GQA causal attention (nn_Attention).

Reference computation (B=2, S=2048, D=4096, H=32, KV=8, HD=128):
    q/k/v projections -> RoPE(q, k) -> GQA attention with additive mask
    -> softmax -> out projection.

Sharding: TP=4 over heads x DP=2 over batch on 8 NeuronCores.
Each core computes, for its batch b and head shard tp:
    Q^T = (x_b @ wq_tp)^T, K^T, V  (projections with RoPE folded via
    host-side even/odd weight-column reordering + on-device rotation)
    S^T = K^T . Q^T per head (scores, transposed layout)
    P^T = exp(S^T - 3) * expmask_tile  (lazy softmax; the -3 shift keeps
          exp and its row sums inside fp16 range and cancels in the
          normalize step)
    U^T = V^T-accumulated P^T; rowsums via DVE tile accumulation + one
          M=128 ones-matmul per (head, chunk) so the per-query reciprocal
          is broadcast-free
    att^T = U^T * (1/rowsum);  out_partial = att @ wo_tp
Host sums the 4 TP partials per batch (the row-parallel all-reduce).

All tensors are fp16 (better mantissa than bf16 at equal speed); matmuls
accumulate in fp32 PSUM. Score tiles are packed in pairs into [128,1024]
fp32 PSUM slots so one ACTIVATE covers up to 1024 columns. Phase B is
software-pipelined one head ahead (scores+exp for head h interleave with
the AV matmuls of head h-1) so ScalarE exp latency never stalls TensorE,
and the out-projection of the previous query chunk fills the pipeline
edges.
"""

import os
import math
import numpy as np

# ---------------------------------------------------------------- constants
B, S, D = 2, 2048, 4096
H, KV, HD = 32, 8, 128
N_REP = H // KV
TP, DP = 4, 2
N_CORES = TP * DP
HL = H // TP            # 8 local q heads
KVL = KV // TP          # 2 local kv heads
P = 128                 # partitions
KT = D // P             # 32 contraction tiles for projections
NJ_FULL = S // 512      # 4 seq chunks of 512
NST = S // P            # 16 seq tiles of 128
HF = np.float16
PSUM_SLOT = 1024        # fp32 elements per 2-bank psS slot
PSUM_BANK = 512         # fp32 elements per PSUM bank
EXP_SHIFT = -3.0        # exp(s + EXP_SHIFT): keeps pt/rowsums in fp16 range

# module-level handle for test harness introspection
last_results = None
_cache = {}


def _classify_mask(mask: np.ndarray):
    """Turn the additive mask into multiplicative per-tile factors.

    Returns (table, uniq) where table[i][j] is 'full' (factor==1
    everywhere), 'zero' (factor==0 everywhere -> tile skipped), or an
    index into uniq, the list of distinct [128,512] f32 factor tiles in
    S^T layout ([sk, sq]).
    """
    m = mask.astype(np.float64)
    rowmax = np.max(m, axis=1, keepdims=True)  # per-query max over keys
    rowmax = np.where(np.isfinite(rowmax), rowmax, 0.0)
    em = np.exp(m - rowmax)                    # [sq, sk] in [0, inf)
    emT = np.ascontiguousarray(em.T).astype(np.float32)  # [sk, sq]
    table = [[None] * NJ_FULL for _ in range(NST)]
    uniq = []
    keys = {}
    for j in range(NJ_FULL):
        first = True
        for i in range(NST):
            t = emT[i * P:(i + 1) * P, j * 512:(j + 1) * 512]
            if np.all(t == 1.0):
                table[i][j] = "full"
                first = False
                continue
            if np.all(t == 0.0):
                table[i][j] = "zero"
                continue
            cols1 = np.all(t == 1.0, axis=0)   # all-ones columns
            cols0 = np.all(t == 0.0, axis=0)   # all-zero columns
            # live range starts after leading all-zero cols (first tile of a
            # j-chunk must start at 0 so the PSUM bank is fully initialized)
            lo = 0
            if not first:
                while lo < 512 and cols0[lo]:
                    lo += 1
            hi = 512
            while hi > lo and cols1[hi - 1]:
                hi -= 1
            w = hi - lo
            sub = t[:, lo:hi]
            key = sub.tobytes()
            if key not in keys:
                keys[key] = len(uniq)
                pad = np.ones((P, 512), np.float32)
                pad[:, :w] = sub
                uniq.append(pad)
            table[i][j] = (lo, w, keys[key])
            first = False
    return table, uniq


def _rope_perm(n_heads):
    """Column permutation putting even rope dims first, odd second, per head."""
    perm = []
    for h in range(n_heads):
        perm += [h * HD + 2 * i for i in range(HD // 2)]
        perm += [h * HD + 2 * i + 1 for i in range(HD // 2)]
    return np.array(perm, dtype=np.int64)


def _pack_groups(tiles):
    """Pack (i, lo) score tiles into [P, PSUM_SLOT] fp32 slots.

    Each tile occupies width 512-lo; a tile may not cross a PSUM bank
    boundary (matmul output must stay within one bank). Returns a list of
    groups; each group is (span, [(i, lo, off), ...]).
    """
    groups = []
    cur = []
    off = 0
    for i, lo in tiles:
        w = 512 - lo
        noff = off
        if (noff // PSUM_BANK) != ((noff + w - 1) // PSUM_BANK):
            noff = ((noff // PSUM_BANK) + 1) * PSUM_BANK
        if noff + w > PSUM_SLOT:
            groups.append((off, cur))
            cur = []
            noff = 0
        cur.append((i, lo, noff))
        off = noff + w
    if cur:
        groups.append((off, cur))
    return groups


def _build(table_sig, table, n_uniq):
    """Build + compile the SPMD Bass program for one mask classification."""
    import concourse.bass as bass
    import concourse.tile as tile
    import concourse.mybir as mybir
    from concourse import bacc

    hf = mybir.dt.float16
    f32 = mybir.dt.float32
    Exp = mybir.ActivationFunctionType.Exp
    MULT = mybir.AluOpType.mult
    ADD = mybir.AluOpType.add

    nc = bacc.Bacc("TRN2", target_bir_lowering=False, debug=False,
                   enable_asserts=False, num_devices=N_CORES)

    xT_d = nc.dram_tensor("xT", [D, S], hf, kind="ExternalInput")
    wq_d = nc.dram_tensor("wq", [D, HL * HD], hf, kind="ExternalInput")
    wk_d = nc.dram_tensor("wk", [D, KVL * HD], hf, kind="ExternalInput")
    wv_d = nc.dram_tensor("wv", [D, KVL * HD], hf, kind="ExternalInput")
    wo_d = nc.dram_tensor("wo", [HL * HD, D], hf, kind="ExternalInput")
    cosf_d = nc.dram_tensor("cosf", [P, S], hf, kind="ExternalInput")
    ssf_d = nc.dram_tensor("ssf", [P, S], hf, kind="ExternalInput")
    ones_d = nc.dram_tensor("ones", [P, P], hf, kind="ExternalInput")
    em_d = [nc.dram_tensor(f"em{u}", [P, 512], hf, kind="ExternalInput")
            for u in range(n_uniq)]
    out_d = nc.dram_tensor("out", [S, D], hf, kind="ExternalOutput")

    with tile.TileContext(nc) as tc:
        with tc.tile_pool(name="consts", bufs=1) as cpool:
            cosf = cpool.tile([P, S], hf, tag="cosf", name="cosf")
            ssf = cpool.tile([P, S], hf, tag="ssf", name="ssf")
            ones = cpool.tile([P, P], hf, tag="ones", name="ones")
            shift = cpool.tile([P, 1], f32, tag="shift", name="shift")
            nc.vector.memset(shift[:], EXP_SHIFT)
            # preload the exp table set during the startup DMA wait so the
            # first real softmax exp doesn't pay the ~2.7us table load
            warm = cpool.tile([P, 1], hf, tag="warm", name="warm")
            nc.scalar.activation(warm[:], shift[:], Exp)
            em_sb = [cpool.tile([P, 512], hf, tag=f"em{u}", name=f"em{u}")
                     for u in range(n_uniq)]

            def load_consts():
                # issued behind the first critical weight loads so they do
                # not delay the first matmul
                nc.sync.dma_start(cosf[:], cosf_d[:, :])
                nc.sync.dma_start(ssf[:], ssf_d[:, :])
                nc.sync.dma_start(ones[:], ones_d[:, :])
                for u in range(n_uniq):
                    nc.sync.dma_start(em_sb[u][:], em_d[u][:, :])

            qkv_pool = tc.alloc_tile_pool(name="qkv", bufs=1)
            QT = [qkv_pool.tile([P, S], hf, tag=f"qt{h}", name=f"qt{h}") for h in range(HL)]
            KTt = [qkv_pool.tile([P, S], hf, tag=f"kt{g}", name=f"kt{g}") for g in range(KVL)]
            V = [qkv_pool.tile([P, KVL * HD], hf, tag=f"v{st}", name=f"v{st}") for st in range(NST)]

            # ------------- phase A: projections + RoPE ------------
            XH = 2 if NJ_FULL % 2 == 0 else 1
            SH = S // XH
            QG = 2 if HL % 2 == 0 else HL   # q heads per weight group
            with tc.tile_pool(name="xt", bufs=1) as xt_pool, \
                 tc.tile_pool(name="wq", bufs=2) as wq_pool, \
                 tc.tile_pool(name="wk", bufs=1) as wk_pool, \
                 tc.tile_pool(name="wv", bufs=1) as wv_pool, \
                 tc.tile_pool(name="ropetmp", bufs=2) as rt_pool, \
                 tc.tile_pool(name="psA", bufs=3, space="PSUM") as psA, \
                 tc.tile_pool(name="psV", bufs=2, space="PSUM") as psV:
                for half in range(XH):
                    s0 = half * SH
                    # V weights first so the first matmul can start as soon
                    # as the first xt column chunk lands.
                    wvt = []
                    for k in range(KT):
                        t = wv_pool.tile([P, KVL * HD], hf, tag=f"wv{k}", name=f"wv{k}")
                        nc.sync.dma_start(t[:], wv_d[k * P:(k + 1) * P, :])
                        wvt.append(t)
                    xt = []
                    for k in range(KT):
                        t = xt_pool.tile([P, SH], hf, tag=f"xt{k}", name=f"xt{k}")
                        xt.append(t)
                    # column-chunked loads: compute on chunk c may start
                    # while chunk c+1 is still in flight (finer first chunks
                    # so the first V matmul starts sooner)
                    chunks = ([(0, 256), (256, 256)] if half == 0 else
                              [(0, 512)])
                    c = chunks[-1][0] + chunks[-1][1]
                    while c < SH:
                        chunks.append((c, 512))
                        c += 512
                    for c, cw in chunks:
                        for k in range(KT):
                            nc.gpsimd.dma_start(
                                xt[k][:, c:c + cw],
                                xT_d[k * P:(k + 1) * P, s0 + c:s0 + c + cw])

                    def rope_gen(dst, wt, coff, jj):
                        """dst[:, jj*512..] = rope((x @ w)[:, coff:coff+128])"""
                        ps = psA.tile([P, 512], f32, tag="psqk", name="psqk")
                        lo = jj * 512 - s0
                        for k in range(KT):
                            nc.tensor.matmul(ps[:], wt[k][:, coff:coff + P],
                                             xt[k][:, lo:lo + 512],
                                             start=(k == 0), stop=(k == KT - 1))
                        qb = rt_pool.tile([P, 512], hf, tag="qb", name="qb")
                        nc.scalar.copy(qb[:], ps[:])
                        qsw = rt_pool.tile([P, 512], hf, tag="qsw", name="qsw")
                        nc.scalar.copy(qsw[0:64, :], ps[64:128, :])
                        nc.scalar.copy(qsw[64:128, :], ps[0:64, :])
                        t1 = rt_pool.tile([P, 512], hf, tag="t1", name="t1")
                        nc.vector.tensor_tensor(
                            t1[:], qb[:], cosf[:, jj * 512:jj * 512 + 512], MULT)
                        t2 = rt_pool.tile([P, 512], hf, tag="t2", name="t2")
                        nc.vector.tensor_tensor(
                            t2[:], qsw[:], ssf[:, jj * 512:jj * 512 + 512], MULT)
                        nc.vector.tensor_tensor(
                            dst[:, jj * 512:jj * 512 + 512], t1[:], t2[:], ADD)

                    half_js = list(range(half * (NJ_FULL // XH),
                                         (half + 1) * (NJ_FULL // XH)))
                    # V and K first so attention can start early
                    for st in range(half * (NST // XH), (half + 1) * (NST // XH)):
                        ps = psV.tile([P, KVL * HD], f32, tag="psv", name="psv")
                        lo = st * P - s0
                        for k in range(KT):
                            nc.tensor.matmul(ps[:], xt[k][:, lo:lo + P], wvt[k][:],
                                             start=(k == 0), stop=(k == KT - 1))
                        nc.scalar.copy(V[st][:], ps[:])
                    wkt = []
                    for k in range(KT):
                        t = wk_pool.tile([P, KVL * HD], hf, tag=f"wk{k}", name=f"wk{k}")
                        nc.sync.dma_start(t[:], wk_d[k * P:(k + 1) * P, :])
                        wkt.append(t)
                    if half == 0:
                        load_consts()
                    for g in range(KVL):
                        for jj in half_js:
                            rope_gen(KTt[g], wkt, g * HD, jj)
                    for hg in range(HL // QG):
                        wqt = []
                        for k in range(KT):
                            t = wq_pool.tile([P, QG * HD], hf, tag=f"wq{k}", name=f"wq{k}")
                            nc.sync.dma_start(
                                t[:], wq_d[k * P:(k + 1) * P,
                                           hg * QG * HD:(hg + 1) * QG * HD])
                            wqt.append(t)
                        for h in range(hg * QG, (hg + 1) * QG):
                            for jj in half_js:
                                rope_gen(QT[h], wqt, (h - hg * QG) * HD, jj)

            # ------------- phase B+C: attention + out projection ------
            ut_pool = tc.alloc_tile_pool(name="ut", bufs=1)
            UT = [ut_pool.tile([P, S], hf, tag=f"ut{h}", name=f"ut{h}")
                  for h in range(HL)]
            wo_pool = tc.alloc_tile_pool(name="wo", bufs=1)
            ob_pool = tc.alloc_tile_pool(name="ob", bufs=2)
            wot = []
            for h in range(HL):
                t = wo_pool.tile([P, D], hf, tag=f"wo{h}", name=f"wo{h}")
                nc.sync.dma_start(t[:], wo_d[h * P:(h + 1) * P, :])
                wot.append(t)
            OBH = 2 if D >= 2048 else 1
            OBW = D // OBH
            NDC = OBW // 512
            with tc.tile_pool(name="pt", bufs=10) as pt_pool, \
                 tc.tile_pool(name="acc", bufs=4) as acc_pool, \
                 tc.tile_pool(name="rnorm", bufs=2) as rn_pool, \
                 tc.tile_pool(name="psS", bufs=2, space="PSUM") as psS_pool, \
                 tc.tile_pool(name="psU", bufs=2, space="PSUM") as psU_pool, \
                 tc.tile_pool(name="psR", bufs=2, space="PSUM") as psR_pool:

                def c_chunk_emitters(st):
                    """One closure per (half, dch) out-proj chunk of seq
                    tile st; each runs 8 accumulating matmuls; ob staging
                    and the output DMA are woven into the first/last
                    chunk of each half."""
                    ems = []
                    state = {}
                    for half in range(OBH):
                        for dch in range(NDC):
                            def emit(st=st, half=half, dch=dch):
                                if dch == 0:
                                    state[half] = ob_pool.tile(
                                        [P, OBW], hf, tag="ob", name="ob")
                                ob = state[half]
                                dc = half * NDC + dch
                                psO = psU_pool.tile([P, 512], f32,
                                                    tag="psu", name="psu")
                                for h in range(HL):
                                    nc.tensor.matmul(
                                        psO[:], UT[h][:, st * P:(st + 1) * P],
                                        wot[h][:, dc * 512:dc * 512 + 512],
                                        start=(h == 0), stop=(h == HL - 1))
                                dsl = slice(dch * 512, dch * 512 + 512)
                                if dch % 2 == 0:
                                    nc.vector.tensor_copy(ob[:, dsl], psO[:])
                                else:
                                    nc.scalar.copy(ob[:, dsl], psO[:])
                                if dch == NDC - 1:
                                    nc.gpsimd.dma_start(
                                        out_d[st * P:(st + 1) * P,
                                              half * OBW:(half + 1) * OBW],
                                        ob[:])
                            ems.append(emit)
                    return ems

                for j in range(NJ_FULL):
                    jsl = slice(j * 512, j * 512 + 512)
                    inc = [i for i in range(NST) if table[i][j] != "zero"]
                    tiles = [(i, 0 if table[i][j] == "full" else table[i][j][0])
                             for i in inc]
                    groups = _pack_groups(tiles)
                    ngr = len(groups)
                    # out-proj chunks of the previous query chunk: first
                    # seq tile interleaves into the k=0 pipeline edge,
                    # the rest run after this chunk's head loop.
                    cfill = c_chunk_emitters(4 * (j - 1)) if j > 0 else []
                    per_step = -(-len(cfill) // ngr) if cfill else 0

                    psUs = {}
                    accs = {}
                    pts = {}
                    lag = 2 if j == 0 else 1
                    for k in range(HL + lag):
                        sc_h = k if k < HL else None
                        av_h = k - lag if k >= lag else None
                        for step in range(ngr):
                            span, grp = groups[step]
                            if sc_h is not None:
                                h = sc_h
                                g = h // N_REP
                                psS = psS_pool.tile([P, PSUM_SLOT], f32,
                                                    tag="pss", name="pss")
                                for (i, lo, off) in grp:
                                    nc.tensor.matmul(
                                        psS[:, off:off + 512 - lo],
                                        KTt[g][:, i * P:(i + 1) * P],
                                        QT[h][:, j * 512 + lo:j * 512 + 512],
                                        start=True, stop=True)
                                pt = pt_pool.tile([P, PSUM_SLOT], hf,
                                                  tag="pt", name="pt")
                                pts[(h, step)] = pt
                                nc.scalar.activation(pt[:, 0:span],
                                                     psS[:, 0:span], Exp,
                                                     bias=shift[:])
                                for (i, lo, off) in grp:
                                    cls = table[i][j]
                                    if cls != "full":
                                        _, w, u = cls
                                        nc.vector.tensor_tensor(
                                            pt[:, off:off + w],
                                            pt[:, off:off + w],
                                            em_sb[u][:, 0:w], MULT)
                                # rowsum accumulation on DVE
                                if step == 0:
                                    acc = acc_pool.tile([P, 512], hf,
                                                        tag="acc", name="acc")
                                    accs[h] = acc
                                acc = accs[h]
                                for (i, lo, off) in grp:
                                    if step == 0 and (i, lo, off) == grp[0]:
                                        nc.vector.tensor_copy(
                                            acc[:], pt[:, off:off + 512])
                                    else:
                                        nc.vector.tensor_tensor(
                                            acc[:, lo:], acc[:, lo:],
                                            pt[:, off:off + 512 - lo], ADD)
                            if av_h is not None:
                                h = av_h
                                g = h // N_REP
                                if step == 0:
                                    psUs[h] = psU_pool.tile(
                                        [P, 512], f32, tag="psu", name="psu")
                                psU = psUs[h]
                                ntile = len(tiles)
                                done = sum(len(gr) for _, gr in groups[:step])
                                for (i, lo, off) in grp:
                                    nc.tensor.matmul(
                                        psU[:, lo:],
                                        V[i][:, g * HD:(g + 1) * HD],
                                        pts[(h, step)][:, off:off + 512 - lo],
                                        start=(done == 0),
                                        stop=(done == ntile - 1))
                                    done += 1
                            if av_h is None and cfill:
                                for _ in range(per_step):
                                    if cfill:
                                        cfill.pop(0)()
                        if av_h is not None:
                            h = av_h
                            # partition-reduce the DVE-accumulated rowsums;
                            # M=128 replicates them so no broadcast needed
                            psR = psR_pool.tile([P, 512], f32,
                                                tag="psr", name="psr")
                            nc.tensor.matmul(psR[:], ones[:, 0:P], accs[h][:],
                                             start=True, stop=True)
                            rb = rn_pool.tile([P, 512], f32, tag="rb", name="rb")
                            nc.vector.reciprocal_approx_fast(rb[:], psR[:])
                            nc.vector.tensor_tensor(UT[h][:, jsl],
                                                    psUs[h][:], rb[:], MULT)
                    for emit in cfill:
                        emit()
                    # remaining out-proj seq tiles of the previous chunk
                    if j > 0:
                        for st in range(4 * (j - 1) + 1, 4 * j):
                            for emit in c_chunk_emitters(st):
                                emit()
                # out projection for the final query chunk
                for st in range(4 * (NJ_FULL - 1), NST):
                    for emit in c_chunk_emitters(st):
                        emit()
            ob_pool.release()
            wo_pool.release()
            ut_pool.release()
            qkv_pool.release()

    nc.compile()
    return nc


def kernel(x, freqs_cos, freqs_sin, mask, wq, wk, wv, wo):
    global last_results
    from concourse.bass_utils import run_bass_kernel_spmd

    x = np.asarray(x)
    mask = np.asarray(mask, dtype=np.float32)
    table, uniq = _classify_mask(mask)
    sig = tuple(tuple(r) for r in table), len(uniq)
    key = ("k", sig)
    if key not in _cache:
        _cache[key] = _build(sig, table, len(uniq))
    nc = _cache[key]

    qperm = _rope_perm(H)
    kperm = _rope_perm(KV)
    wq_r = np.asarray(wq)[:, qperm]
    wk_r = (np.asarray(wk) * (1.0 / math.sqrt(HD)))[:, kperm]
    wv_n = np.asarray(wv)
    wo_n = np.asarray(wo)

    cosT = np.asarray(freqs_cos).T.astype(np.float32)     # [64, S]
    sinT = np.asarray(freqs_sin).T.astype(np.float32)
    cosf = np.concatenate([cosT, cosT], axis=0).astype(HF)  # [128, S]
    ssf = np.concatenate([-sinT, sinT], axis=0).astype(HF)
    ones = np.ones((P, P), dtype=HF)

    in_maps = []
    for c in range(N_CORES):
        b, tp = c // TP, c % TP
        m = {
            "xT": np.ascontiguousarray(x[b].T).astype(HF),
            "wq": np.ascontiguousarray(wq_r[:, tp * HL * HD:(tp + 1) * HL * HD]).astype(HF),
            "wk": np.ascontiguousarray(wk_r[:, tp * KVL * HD:(tp + 1) * KVL * HD]).astype(HF),
            "wv": np.ascontiguousarray(wv_n[:, tp * KVL * HD:(tp + 1) * KVL * HD]).astype(HF),
            "wo": np.ascontiguousarray(wo_n[tp * HL * HD:(tp + 1) * HL * HD, :]).astype(HF),
            "cosf": cosf, "ssf": ssf, "ones": ones,
        }
        for u, t in enumerate(uniq):
            m[f"em{u}"] = t.astype(HF)
        in_maps.append(m)

    trace = bool(os.environ.get("BASS_TRACE"))
    try:
        import antenv.axon_hooks  # noqa: F401
    except ImportError:
        # bass_utils re-reads BASS_TRACE internally and would crash on the
        # missing NTFF hook module; fall back to an untraced run instead.
        os.environ.setdefault("BASS_NEVER_TRACE", "1")
        trace = False
    last_results = run_bass_kernel_spmd(
        nc, in_maps, core_ids=list(range(N_CORES)), trace=trace)

    out = np.zeros((B, S, D), dtype=np.float32)
    for c in range(N_CORES):
        out[c // TP] += last_results.results[c]["out"]
    return out
